# revision 11
# baseline (speedup 1.0000x reference)
"""Trainium2 Bass kernel for nn_Attention_83004537963197.

LayerNorm -> QKV projection -> 8-head attention (head_dim=16) -> output
projection, x[16, 1024, 1024] f32.  Data-parallel over batch: 2 batches
per NeuronCore across 8 cores, no collectives.

v2 highlights (vs v1 at ~282us):
  - One activation table set for the entire kernel: LN rstd is computed
    as exp(-0.5*ln(var+eps)) and the act-table list is filtered so Ln
    and Exp both resolve to natural_log_exp_and_others (v1 thrashed
    exp<->sqrt ACT_TABLE_LOADs, 2.7us each, mid-exp-stream).
  - x^T is produced by HWDGE DMA-transpose (Xbar) instead of PE matmuls
    against an identity: saves 64 matmuls + 16 LDWEIGHTS + ~22us of DVE
    PSUM-evacuation per core, and the ramp's transposes ride the
    otherwise-idle scalar queue.
  - exp ACTIVATEs are double-buffered [P,2,512]f32 score tiles (PSUM
    banks 0-3) so ScalarE runs back-to-back; attnv accumulates into oT
    (banks 4-5, one per (r, ih) group, rotating), small pool banks 6-7.
  - Softmax row-sums via the ones-column trick (v_aug col 0 = 1.0);
    normalize evacuates oT with ONE full-width DVE copy (v1: four
    single-lane copies), reciprocal on [P,16], stride-0 DRAM broadcast.
  - q^T/k^T relocation DMAs are full-n [16, 1024] (16/batch, half of
    v1's count), on the scalar queue for batch 0 (idle during the ramp)
    and the gpsimd queue for batch 1.
  - Final output stored bf16 (host upcasts): halves write traffic.
  - Schedule: per chunk the attention-critical matmuls are emitted
    first, prep/projection fillers behind them in the engine FIFOs,
    with pop positions tuned so no filler's dependency can stall the
    PE queue ahead of the exp stream.
"""

from contextlib import ExitStack

import numpy as np
import ml_dtypes

import concourse.bass as bass
import concourse.tile as tile
from concourse import bacc, mybir, hw_specs
from concourse.bass_utils import run_bass_kernel_spmd

# ---- single activation-table-set patch -------------------------------
# The act-table placement pass maps each activation to the first table
# set containing its function (Exp -> exp_and_others, Ln -> natural_log)
# which thrashes 2.7us ACT_TABLE_LOADs when they interleave.  Restrict
# Exp/Ln to the one set holding both so the kernel needs exactly one
# load.  (List order/indices are preserved for act_func_set_id.)
_orig_get_act_tables = hw_specs.get_activation_tables


def _patched_get_act_tables(arch):
    tabs = _orig_get_act_tables(arch)
    EXP = mybir.ActivationFunctionType.Exp
    LN = mybir.ActivationFunctionType.Ln
    out = {}
    for name, funcs in tabs.items():
        if name != "natural_log_exp_and_others":
            funcs = funcs - {EXP, LN}
        out[name] = funcs
    return out


hw_specs.get_activation_tables = _patched_get_act_tables
import concourse.bacc as _bacc_mod  # noqa: E402

_bacc_mod.get_activation_tables = _patched_get_act_tables

F32 = mybir.dt.float32
BF16 = mybir.dt.bfloat16

B, N, EMB = 16, 1024, 1024
HEADS, INNER = 8, 128
HD = INNER // HEADS            # 16
SCALE = INNER ** -0.5
EPS = 1e-5
NCORES = 8
NB = B // NCORES               # batches per core
P = 128
NT = EMB // P                  # 8 tiles along emb / n

Sub = mybir.AluOpType.subtract
Mult = mybir.AluOpType.mult
Add = mybir.AluOpType.add
AF = mybir.ActivationFunctionType

_CACHE = {}


def _build():
    nc = bacc.Bacc(None, target_bir_lowering=False)

    xs_h = nc.declare_dram_parameter("xs", [NB, N, EMB], F32, isOutput=False)
    wqk_h = nc.declare_dram_parameter("wqk", [P, NT, 2, P], BF16, isOutput=False)
    bqk_h = nc.declare_dram_parameter("bqk", [P, 2], F32, isOutput=False)
    wv_h = nc.declare_dram_parameter("wv", [P, NT, P], BF16, isOutput=False)
    bv_h = nc.declare_dram_parameter("bv", [1, P], BF16, isOutput=False)
    wpj_h = nc.declare_dram_parameter("wproj", [P, 2, EMB], BF16, isOutput=False)
    out_h = nc.declare_dram_parameter("out", [NB, N, EMB], BF16, isOutput=True)

    with tile.TileContext(nc) as tc, ExitStack() as ctx:
        ent = ctx.enter_context
        const = ent(tc.tile_pool(name="const", bufs=1))
        xpool = ent(tc.tile_pool(name="xpool", bufs=8))
        xnpool = ent(tc.tile_pool(name="xnpool", bufs=3))
        stat = ent(tc.tile_pool(name="stat", bufs=4))
        xT_pool = ent(tc.tile_pool(name="xT", bufs=2))
        qk_pool = ent(tc.tile_pool(name="qk", bufs=2))
        v_pool = ent(tc.tile_pool(name="vp", bufs=2))
        e_pool = ent(tc.tile_pool(name="ep", bufs=3))
        o_pool = ent(tc.tile_pool(name="op", bufs=2))
        nrm_pool = ent(tc.tile_pool(name="nrm", bufs=2))
        fin_pool = ent(tc.tile_pool(name="fin", bufs=4))
        f1_pool = ent(tc.tile_pool(name="f1", bufs=1))
        dram_pool = ent(tc.tile_pool(name="dsc", bufs=2, space="DRAM"))
        # PSUM: exactly 8 banks
        ps_scores = ent(tc.tile_pool(name="psc", bufs=2, space="PSUM"))  # 0-3
        ps_out = ent(tc.tile_pool(name="pso", bufs=2, space="PSUM"))     # 4-5
        ps_small = ent(tc.tile_pool(name="pss", bufs=2, space="PSUM"))   # 6-7

        # ---- constants ----
        wqk_sb = const.tile([P, NT, 2, P], BF16)
        nc.sync.dma_start(out=wqk_sb, in_=wqk_h[:])
        bqk_sb = const.tile([P, 2], F32)
        nc.sync.dma_start(out=bqk_sb, in_=bqk_h[:])
        wv_sb = const.tile([P, NT, P], BF16)
        nc.sync.dma_start(out=wv_sb, in_=wv_h[:])
        bv_sb = const.tile([1, P], BF16)
        nc.sync.dma_start(out=bv_sb, in_=bv_h[:])
        wpj_sb = const.tile([P, 2, EMB], BF16)
        nc.sync.dma_start(out=wpj_sb, in_=wpj_h[:])
        eps_sb = const.tile([P, 1], F32)
        nc.vector.memset(eps_sb, EPS)
        ones1_sb = const.tile([1, P], BF16)
        nc.vector.memset(ones1_sb, 1.0)

        st8 = {0: {}, 1: {}}   # per-batch live tiles

        # ---------------- prep: LN / transpose / qkv / v ----------------

        def _state(b):
            s = st8[b]
            if s.get("xT") is None:
                s["xT"] = xT_pool.tile([P, NT, N], BF16, tag="xTt", name="xTt")
                s["xraw"] = [None] * NT
                s["mv4"] = [None, None]
                s["rstd4"] = [None, None]
            return s

        def emit_x_load(b, it, q=0):
            s = _state(b)
            xt = xpool.tile([P, EMB], F32, tag="xt", name="xt")
            eng = nc.sync if q == 0 else nc.gpsimd
            eng.dma_start(out=xt, in_=xs_h[b, it * P:(it + 1) * P, :])
            s["xraw"][it] = xt

        def emit_ln_stats(b, it):
            s = _state(b)
            g, k = it // 4, it % 4
            if k == 0:
                s["mv4"][g] = stat.tile([P, 4, 2], F32, tag="mv4", name="mv4")
                s["rstd4"][g] = stat.tile([P, 4], F32, tag="rstd4", name="rstd4")
            xt = s["xraw"][it]
            st = stat.tile([P, 2, 6], F32, tag="st")
            nc.vector.bn_stats(out=st[:, 0, :], in_=xt[:, 0:512])
            nc.vector.bn_stats(out=st[:, 1, :], in_=xt[:, 512:1024])
            nc.vector.bn_aggr(out=s["mv4"][g][:, k, :], in_=st)

        def emit_rstd(b, g):
            # rstd for 4 it-tiles: exp(-0.5*ln(var+eps)) -- stays in the
            # natural_log_exp_and_others set with the attention exp
            s = st8[b]
            lnv = stat.tile([P, 4], F32, tag="lnv")
            nc.scalar.activation(out=lnv, in_=s["mv4"][g][:, :, 1],
                                 func=AF.Ln, bias=eps_sb)
            nc.scalar.activation(out=s["rstd4"][g], in_=lnv,
                                 func=AF.Exp, scale=-0.5)

        def emit_ln_norm(b, it):
            s = st8[b]
            g, k = it // 4, it % 4
            xn = xnpool.tile([P, EMB], BF16, tag="xn")
            nc.vector.tensor_scalar(
                out=xn, in0=s["xraw"][it], scalar1=s["mv4"][g][:, k, 0:1],
                scalar2=s["rstd4"][g][:, k:k + 1], op0=Sub, op1=Mult)
            s["xraw"][it] = xn     # replaced by normalized bf16

        def emit_tp(b, it):
            # x^T via HWDGE Xbar DMA-transpose (scalar queue for batch 0's
            # ramp, sync queue for batch 1 to keep the exp stream clean)
            s = st8[b]
            eng = nc.scalar if b == 0 else nc.sync
            eng.dma_start_transpose(
                out=s["xT"][:, :, it * P:(it + 1) * P], in_=s["xraw"][it])

        def emit_qk_chunk(b, t, nt):
            # compact q^T/k^T halves [128 rows = 8 heads x 16, 512 n];
            # after the second half: relocate per-head rows into the
            # 32-aligned region layout with 8 full-n [16, 1024] DMAs.
            s = st8[b]
            if s.get("qkc") is None:
                s["qkc"] = qk_pool.tile([P, 2, N], BF16, tag="qkc", name="qkc")
                s["qT"] = qk_pool.tile([P, 2, N], BF16, tag="qT", name="qT")
                s["kT"] = qk_pool.tile([P, 2, N], BF16, tag="kT", name="kT")
            xT = s["xT"]
            ps = ps_small.tile([P, 512], F32, tag="smallps")
            for et in range(NT):
                nc.tensor.matmul(
                    ps, wqk_sb[:, et, t, :],
                    xT[:, et, nt * 512:(nt + 1) * 512],
                    start=(et == 0), stop=(et == NT - 1))
            nc.vector.tensor_scalar(
                out=s["qkc"][:, t, nt * 512:(nt + 1) * 512], in0=ps,
                scalar1=bqk_sb[:, t:t + 1], scalar2=None, op0=Add)
            if nt == 1:
                dst = s["qT"] if t == 0 else s["kT"]
                eng = nc.scalar if b == 0 else nc.gpsimd
                for h in range(HEADS):
                    r, c = h // 4, h % 4
                    eng.dma_start(
                        out=dst[32 * c:32 * c + HD, r, :],
                        in_=s["qkc"][HD * h:HD * (h + 1), t, :])

        def emit_v_chunk(b, jt):
            s = st8[b]
            if s.get("v") is None:
                s["v"] = v_pool.tile([P, NT, HEADS, 32], BF16, tag="vt", name="vt")
                nc.gpsimd.memset(s["v"], 0.0)
                nc.gpsimd.memset(s["v"][:, :, :, 0:1], 1.0)
            xT = s["xT"]
            ps = ps_small.tile([P, P], F32, tag="smallps")
            for et in range(NT):
                nc.tensor.matmul(
                    ps, xT[:, et, jt * P:(jt + 1) * P], wv_sb[:, et, :],
                    start=(et == 0), stop=False)
            nc.tensor.matmul(ps, ones1_sb, bv_sb, start=False, stop=True)
            nc.vector.tensor_copy(
                out=s["v"][:, jt, :, 1:17],
                in_=ps[:].rearrange("p (h d) -> p h d", d=16))

        # ---------------- projection ----------------

        def emit_proj1(b, it, nt):
            # region-0 half of the projection, stashed in SBUF bf16
            s = st8[b]
            if s.get("fin1") is None:
                s["fin1"] = f1_pool.tile([P, NT, 2, 512], BF16,
                                         tag="fin1", name="fin1")
            ps = ps_small.tile([P, 512], F32, tag="smallps")
            nc.tensor.matmul(
                ps, s["o"][0][:, it * P:(it + 1) * P],
                wpj_sb[:, 0, nt * 512:(nt + 1) * 512],
                start=True, stop=True)
            nc.vector.tensor_copy(out=s["fin1"][:, it, nt, :], in_=ps)

        def emit_proj2(b, it, nt):
            s = st8[b]
            ps = ps_small.tile([P, 512], F32, tag="smallps")
            nc.tensor.matmul(
                ps, s["o"][1][:, it * P:(it + 1) * P],
                wpj_sb[:, 1, nt * 512:(nt + 1) * 512],
                start=True, stop=True)
            fin = fin_pool.tile([P, 512], BF16, tag="fin")
            nc.vector.tensor_add(fin, s["fin1"][:, it, nt, :], ps)
            nc.sync.dma_start(
                out=out_h[b, it * P:(it + 1) * P, nt * 512:(nt + 1) * 512],
                in_=fin)

        # ---------------- attention ----------------

        def emit_normalize(b, r, ih, oT_ps):
            # oT_ps [P, 512] f32: rows 32c = softmax row sums (ones-column
            # trick).  One full-tile evacuation, reciprocal on [P,16],
            # stride-0 DRAM broadcast, then one multiply.
            s = st8[b]
            if s["o"][r] is None:
                s["o"][r] = o_pool.tile([P, N], BF16, tag="oT", name="oT")
            i0 = ih * 512
            srow = nrm_pool.tile([P, 512], F32, tag="srow")
            nc.vector.tensor_copy(out=srow, in_=oT_ps)
            scr1 = dram_pool.tile([4, 512], F32, tag="scr1")
            nc.gpsimd.dma_start(out=scr1, in_=srow[0::32, :])
            cmp = nrm_pool.tile([P, 16], F32, tag="cmp")
            flat = scr1[:].rearrange("a (pp cc) -> (a pp) cc", cc=16)
            nc.gpsimd.dma_start(out=cmp, in_=flat)
            rec = nrm_pool.tile([P, 16], F32, tag="rec")
            nc.vector.reciprocal(out=rec, in_=cmp)
            scr2 = dram_pool.tile([4, 512], F32, tag="scr2")
            nc.gpsimd.dma_start(
                out=scr2[:].rearrange("a (pp cc) -> (a pp) cc", cc=16),
                in_=rec)
            rep = nrm_pool.tile([P, 512], F32, tag="rep")
            for c in range(4):
                src = scr2[c:c + 1, :]
                bcast = bass.AP(
                    tensor=src.tensor, offset=src.offset,
                    ap=[[0, 32]] + list(src.ap[1:]))
                nc.gpsimd.dma_start(
                    out=rep[32 * c:32 * c + 32, :], in_=bcast)
            nc.vector.tensor_mul(s["o"][r][:, i0:i0 + 512], oT_ps, rep)

        def emit_attention(b, fillers):
            # group (r, ih); chunk (jt, cpair): scores (2 row-tiled MMs,
            # K=16) -> sc [P,2,512]f32 (double-buffered) -> one N=1024 exp
            # -> E bf16 -> 2 col-tiled attnv MMs accumulating into oT.
            s = st8[b]
            s["o"] = [None, None]

            def fill():
                if fillers:
                    f = fillers.pop(0)
                    if f is not None:
                        f()

            for r in range(2):
                for ih in range(2):
                    oT_ps = ps_out.tile([P, 512], F32, tag="oTps")
                    i0 = ih * 512
                    for jt in range(NT):
                        for cp in range(2):
                            c0 = 2 * cp
                            sc = ps_scores.tile([P, 2, 512], F32, tag="sc")
                            for ci in range(2):
                                c = c0 + ci
                                nc.tensor.matmul(
                                    sc[:, ci, :],
                                    s["kT"][32 * c:32 * c + HD, r,
                                            jt * P:(jt + 1) * P],
                                    s["qT"][32 * c:32 * c + HD, r,
                                            i0:i0 + 512],
                                    start=True, stop=True,
                                    tile_position=(32 * c, 0))
                            E = e_pool.tile([P, 2, 512], BF16, tag="E")
                            nc.scalar.activation(out=E, in_=sc, func=AF.Exp)
                            for ci in range(2):
                                c = c0 + ci
                                h = 4 * r + c
                                nc.tensor.matmul(
                                    oT_ps[32 * c:32 * c + 32, :],
                                    s["v"][:, jt, h, :], E[:, ci, :],
                                    start=(jt == 0), stop=(jt == NT - 1),
                                    tile_position=(0, 32 * c))
                            fill()
                    emit_normalize(b, r, ih, oT_ps)

        # ---------------- schedule ----------------
        # preload the (single) act table while the first DMAs run
        dummy = stat.tile([P, 1], F32, tag="dummy")
        nc.scalar.activation(out=dummy, in_=eps_sb, func=AF.Exp)

        # ---- batch 0 prep (the ramp) ----
        for it in range(NT):
            emit_x_load(0, it, q=it % 2)
        for it in range(4):
            emit_ln_stats(0, it)
        emit_rstd(0, 0)
        for it in range(4):
            emit_ln_norm(0, it)
            emit_tp(0, it)
        for it in range(4, NT):
            emit_ln_stats(0, it)
        emit_rstd(0, 1)
        for it in range(4, NT):
            emit_ln_norm(0, it)
            emit_tp(0, it)
        emit_qk_chunk(0, 1, 0)   # k first (scores stationary), reloc at nt=1
        emit_qk_chunk(0, 1, 1)
        emit_qk_chunk(0, 0, 0)
        emit_qk_chunk(0, 0, 1)
        for jt in range(NT):
            emit_v_chunk(0, jt)

        # ---- fillers for attention(0): batch-1 prep + proj1(0) ----
        # attention(0) has 64 chunks, one filler pop per chunk.
        # groups: (r0,ih0) ch 1-16, (r0,ih1) 17-32, (r1,ih0) 33-48,
        # (r1,ih1) 49-64.  o[0] halves exist after ch 16/32 + normalize.
        fill_a0 = []
        for it in range(4):
            fill_a0.append(lambda it=it: emit_x_load(1, it, q=it % 2))
        for it in range(4):
            fill_a0.append(lambda it=it: emit_ln_stats(1, it))
        fill_a0.append(lambda: emit_rstd(1, 0))
        for it in range(2):
            fill_a0.append(lambda it=it: emit_x_load(1, it + 4, q=it % 2))
        for it in range(4):
            fill_a0.append(lambda it=it: emit_ln_norm(1, it))
            fill_a0.append(lambda it=it: emit_tp(1, it))
        for it in range(2):
            fill_a0.append(lambda it=it: emit_x_load(1, it + 6, q=it % 2))
        for it in range(4, NT):
            fill_a0.append(lambda it=it: emit_ln_stats(1, it))
        fill_a0.append(lambda: emit_rstd(1, 1))
        for it in range(4, NT):
            fill_a0.append(lambda it=it: emit_ln_norm(1, it))
            fill_a0.append(lambda it=it: emit_tp(1, it))
        fill_a0 += [lambda: emit_qk_chunk(1, 1, 0),
                    lambda: emit_qk_chunk(1, 1, 1),
                    lambda: emit_qk_chunk(1, 0, 0),
                    lambda: emit_qk_chunk(1, 0, 1)]
        fill_a0 += [lambda jt=jt: emit_v_chunk(1, jt) for jt in range(NT)]
        # 46 so far; proj1(0) it 0-3 pops ~47-54 (valid ~ch 19),
        # it 4-7 pops ~55-62 (valid ~ch 35)
        fill_a0 += [lambda it=it, nt=nt: emit_proj1(0, it, nt)
                    for it in range(NT) for nt in range(2)]

        emit_attention(0, fill_a0)
        for f in fill_a0:
            if f is not None:
                f()

        # ---- attention 1 fillers; pop positions respect o(1) readiness --
        fill_a1 = (
            [lambda it=it, nt=nt: emit_proj2(0, it, nt)
             for it in range(NT) for nt in range(2)]          # pops 1-16
            + [None] * 2
            + [lambda it=it, nt=nt: emit_proj1(1, it, nt)
               for it in range(4) for nt in range(2)]         # 19-26
            + [None] * 8
            + [lambda it=it, nt=nt: emit_proj1(1, it, nt)
               for it in range(4, NT) for nt in range(2)]     # 35-42
            + [None] * 10
            + [lambda it=it, nt=nt: emit_proj2(1, it, nt)
               for it in range(4) for nt in range(2)]         # 53-60
        )
        emit_attention(1, fill_a1)
        for f in fill_a1:
            if f is not None:
                f()
        for it in range(4, NT):
            for nt in range(2):
                emit_proj2(1, it, nt)

    nc.finalize()
    return nc


def _prep_weights(gamma, beta, w_qkv, w_proj, b_proj):
    gamma = gamma.astype(np.float64)
    beta = beta.astype(np.float64)
    w_qkv = w_qkv.astype(np.float64)
    w_proj = w_proj.astype(np.float64)
    b_proj = b_proj.astype(np.float64)

    wg = w_qkv * gamma[:, None]
    bias = beta @ w_qkv                   # [384]

    # compact q/k: tile t=0 -> q (SCALE folded), t=1 -> k
    wqk = np.zeros((EMB, 2, P), dtype=np.float64)
    wqk[:, 0, :] = wg[:, :INNER] * SCALE
    wqk[:, 1, :] = wg[:, INNER:2 * INNER]
    bqk = np.zeros((P, 2), dtype=np.float64)
    bqk[:, 0] = bias[:INNER] * SCALE
    bqk[:, 1] = bias[INNER:2 * INNER]
    wqk_t = wqk.reshape(NT, P, 2, P).transpose(1, 0, 2, 3)  # [P, NT, 2, P]

    wv = wg[:, 2 * INNER:3 * INNER].reshape(NT, P, P).transpose(1, 0, 2)
    bv = bias[2 * INNER:3 * INNER].reshape(1, P)

    # o^T row mapping: 32c = ones/rowsum row, 32c+1+d = head (4r+c) dim d
    wpj = np.zeros((P, 2, EMB), dtype=np.float64)
    for r in range(2):
        for c in range(4):
            h = 4 * r + c
            wpj[32 * c + 1:32 * c + 1 + HD, r, :] = \
                w_proj[h * HD:(h + 1) * HD, :]
    wpj[0, 0, :] = b_proj

    bf = ml_dtypes.bfloat16
    return {
        "wqk": np.ascontiguousarray(wqk_t).astype(bf),
        "bqk": np.ascontiguousarray(bqk).astype(np.float32),
        "wv": np.ascontiguousarray(wv).astype(bf),
        "bv": np.ascontiguousarray(bv).astype(bf),
        "wproj": np.ascontiguousarray(wpj).astype(bf),
    }


def kernel(x, gamma, beta, w_qkv, w_proj, b_proj):
    if "nc" not in _CACHE:
        _CACHE["nc"] = _build()
    nc = _CACHE["nc"]

    w = _prep_weights(gamma, beta, w_qkv, w_proj, b_proj)
    x = np.asarray(x, dtype=np.float32)
    in_maps = []
    for i in range(NCORES):
        m = {"xs": np.ascontiguousarray(x[i * NB:(i + 1) * NB])}
        m.update(w)
        in_maps.append(m)

    res = run_bass_kernel_spmd(nc, in_maps, core_ids=list(range(NCORES)))
    out = np.concatenate([res.results[i]["out"] for i in range(NCORES)], axis=0)
    return out.astype(np.float32)


# revision 14
# speedup vs baseline: 1.0012x; 1.0012x over previous
"""Trainium2 Bass kernel for nn_Attention_83004537963197.

LayerNorm -> QKV projection -> 8-head attention (head_dim=16) -> output
projection, x[16, 1024, 1024] f32.  Data-parallel over batch: 2 batches
per NeuronCore across 8 cores, no collectives.

v3 structure:
  - One activation table set for the entire kernel: LN rstd is computed
    as exp(-0.5*ln(var+eps)) and the act-table list is filtered so Ln
    and Exp both resolve to natural_log_exp_and_others.
  - x^T via HWDGE Xbar DMA-transpose (no PE/identity matmuls, no DVE
    PSUM evacuation); batch-0 transposes on the scalar queue (idle
    during the ramp), batch-1 on the sync queue.
  - Ramp is a per-tile pipeline: x loads lead the sync/gpsimd queues
    (weights ride the vector queue), each it-tile runs
    stats -> per-tile rstd -> normalize -> transpose independently, and
    q/k relocation happens per (t, nt)-half on two parallel queues so
    the first attention chunk fires as soon as plane r0/i-half-0 is up.
  - Attention groups ordered (r0,ih0),(r1,ih0),(r0,ih1),(r1,ih1): both
    regions' outputs for an i-half complete together, so the output
    projection is a single pass (2 accumulating matmuls per chunk, no
    region split/stash), and the serial tail is only the last i-half's
    8 projection chunks.
  - exp ACTIVATEs N=1024 from double-buffered [P,2,512]f32 score tiles
    (PSUM banks 0-3; oT banks 4-5; small pool 6-7), scores 2-way
    row-tiled (K=16 at 32c), attnv 2-way col-tiled, softmax row-sums
    via the ones-column trick, reciprocal+stride-0-broadcast normalize
    with a single full-width PSUM evacuation.
  - Output stored bf16 (host upcasts).
"""

from contextlib import ExitStack

import numpy as np
import ml_dtypes

import concourse.bass as bass
import concourse.tile as tile
from concourse import bacc, mybir, hw_specs
from concourse.bass_utils import run_bass_kernel_spmd

# ---- single activation-table-set patch -------------------------------
# The act-table placement pass maps each activation to the first table
# set containing its function (Exp -> exp_and_others, Ln -> natural_log)
# which thrashes 2.7us ACT_TABLE_LOADs when they interleave.  Restrict
# Exp/Ln to the one set holding both so the kernel needs exactly one
# load.  (List order/indices are preserved for act_func_set_id.)
_orig_get_act_tables = hw_specs.get_activation_tables


def _patched_get_act_tables(arch):
    tabs = _orig_get_act_tables(arch)
    EXP = mybir.ActivationFunctionType.Exp
    LN = mybir.ActivationFunctionType.Ln
    out = {}
    for name, funcs in tabs.items():
        if name != "natural_log_exp_and_others":
            funcs = funcs - {EXP, LN}
        out[name] = funcs
    return out


hw_specs.get_activation_tables = _patched_get_act_tables
import concourse.bacc as _bacc_mod  # noqa: E402

_bacc_mod.get_activation_tables = _patched_get_act_tables

F32 = mybir.dt.float32
BF16 = mybir.dt.bfloat16

B, N, EMB = 16, 1024, 1024
HEADS, INNER = 8, 128
HD = INNER // HEADS            # 16
SCALE = INNER ** -0.5
EPS = 1e-5
NCORES = 8
NB = B // NCORES               # batches per core
P = 128
NT = EMB // P                  # 8 tiles along emb / n

Sub = mybir.AluOpType.subtract
Mult = mybir.AluOpType.mult
Add = mybir.AluOpType.add
AF = mybir.ActivationFunctionType

_CACHE = {}


def _build():
    nc = bacc.Bacc(None, target_bir_lowering=False)

    xs_h = nc.declare_dram_parameter("xs", [NB, N, EMB], F32, isOutput=False)
    wqk_h = nc.declare_dram_parameter("wqk", [P, NT, 2, P], BF16, isOutput=False)
    bqk_h = nc.declare_dram_parameter("bqk", [P, 2], F32, isOutput=False)
    wv_h = nc.declare_dram_parameter("wv", [P, NT, P], BF16, isOutput=False)
    bv_h = nc.declare_dram_parameter("bv", [1, P], BF16, isOutput=False)
    wpj_h = nc.declare_dram_parameter("wproj", [P, 2, EMB], BF16, isOutput=False)
    out_h = nc.declare_dram_parameter("out", [NB, N, EMB], BF16, isOutput=True)

    with tile.TileContext(nc) as tc, ExitStack() as ctx:
        ent = ctx.enter_context
        const = ent(tc.tile_pool(name="const", bufs=1))
        xpool = ent(tc.tile_pool(name="xpool", bufs=8))
        xnpool = ent(tc.tile_pool(name="xnpool", bufs=3))
        stat = ent(tc.tile_pool(name="stat", bufs=4))
        xT_pool = ent(tc.tile_pool(name="xT", bufs=2))
        qk_pool = ent(tc.tile_pool(name="qk", bufs=2))
        v_pool = ent(tc.tile_pool(name="vp", bufs=2))
        e_pool = ent(tc.tile_pool(name="ep", bufs=4))
        o_pool = ent(tc.tile_pool(name="op", bufs=2))
        nrm_pool = ent(tc.tile_pool(name="nrm", bufs=2))
        fin_pool = ent(tc.tile_pool(name="fin", bufs=4))
        dram_pool = ent(tc.tile_pool(name="dsc", bufs=2, space="DRAM"))
        # PSUM: exactly 8 banks
        ps_scores = ent(tc.tile_pool(name="psc", bufs=2, space="PSUM"))  # 0-3
        ps_out = ent(tc.tile_pool(name="pso", bufs=2, space="PSUM"))     # 4-5
        ps_small = ent(tc.tile_pool(name="pss", bufs=2, space="PSUM"))   # 6-7

        # ---- constants (scalar queue: sync/gpsimd lead with x tiles) ----
        wqk_sb = const.tile([P, NT, 2, P], BF16)
        nc.scalar.dma_start(out=wqk_sb, in_=wqk_h[:])
        bqk_sb = const.tile([P, 2], F32)
        nc.scalar.dma_start(out=bqk_sb, in_=bqk_h[:])
        wv_sb = const.tile([P, NT, P], BF16)
        nc.scalar.dma_start(out=wv_sb, in_=wv_h[:])
        bv_sb = const.tile([1, P], BF16)
        nc.scalar.dma_start(out=bv_sb, in_=bv_h[:])
        wpj_sb = const.tile([P, 2, EMB], BF16)
        nc.scalar.dma_start(out=wpj_sb, in_=wpj_h[:])
        eps_sb = const.tile([P, 1], F32)
        nc.vector.memset(eps_sb, EPS)
        ones1_sb = const.tile([1, P], BF16)
        nc.vector.memset(ones1_sb, 1.0)

        st8 = {0: {}, 1: {}}   # per-batch live tiles

        # ---------------- prep: LN / transpose / qkv / v ----------------

        def _state(b):
            s = st8[b]
            if s.get("xT") is None:
                s["xT"] = xT_pool.tile([P, NT, N], BF16, tag="xTt", name="xTt")
                s["xraw"] = [None] * NT
                s["mv4"] = [None, None]
                s["rstd4"] = [None, None]
            return s

        def emit_x_load(b, it, q=0):
            s = _state(b)
            xt = xpool.tile([P, EMB], F32, tag="xt", name="xt")
            eng = nc.sync if q == 0 else nc.gpsimd
            eng.dma_start(out=xt, in_=xs_h[b, it * P:(it + 1) * P, :])
            s["xraw"][it] = xt

        def emit_ln_stats(b, it):
            s = _state(b)
            g, k = it // 4, it % 4
            if k == 0:
                s["mv4"][g] = stat.tile([P, 4, 2], F32, tag="mv4", name="mv4")
                s["rstd4"][g] = stat.tile([P, 4], F32, tag="rstd4", name="rstd4")
            xt = s["xraw"][it]
            st = stat.tile([P, 2, 6], F32, tag="st")
            nc.vector.bn_stats(out=st[:, 0, :], in_=xt[:, 0:512])
            nc.vector.bn_stats(out=st[:, 1, :], in_=xt[:, 512:1024])
            nc.vector.bn_aggr(out=s["mv4"][g][:, k, :], in_=st)

        def emit_rstd(b, g, k=None):
            # rstd = exp(-0.5*ln(var+eps)); per-tile (k given) during the
            # ramp when ScalarE is idle, batched per-4 as an attention
            # filler so the exp stream only pays ~0.6us per batch of 4.
            s = st8[b]
            sl = slice(0, 4) if k is None else slice(k, k + 1)
            n = 4 if k is None else 1
            lnv = stat.tile([P, 4], F32, tag="lnv")
            nc.scalar.activation(out=lnv[:, sl], in_=s["mv4"][g][:, sl, 1],
                                 func=AF.Ln, bias=eps_sb)
            nc.scalar.activation(out=s["rstd4"][g][:, sl], in_=lnv[:, sl],
                                 func=AF.Exp, scale=-0.5)

        def emit_ln_norm(b, it):
            s = st8[b]
            g, k = it // 4, it % 4
            xn = xnpool.tile([P, EMB], BF16, tag="xn")
            nc.vector.tensor_scalar(
                out=xn, in0=s["xraw"][it], scalar1=s["mv4"][g][:, k, 0:1],
                scalar2=s["rstd4"][g][:, k:k + 1], op0=Sub, op1=Mult)
            s["xraw"][it] = xn     # replaced by normalized bf16

        def emit_tp(b, it):
            # x^T via HWDGE Xbar DMA-transpose (scalar queue for batch 0's
            # ramp, sync queue for batch 1 to keep the exp stream clean)
            s = st8[b]
            eng = nc.scalar if b == 0 else nc.sync
            eng.dma_start_transpose(
                out=s["xT"][:, :, it * P:(it + 1) * P], in_=s["xraw"][it])

        def emit_qk_chunk(b, t, nt):
            # compact q^T/k^T half [128 rows = 8 heads x 16, 512 n], then
            # relocate this half's head rows into the 32-aligned region
            # layout (8 [16,512] DMAs; k on scalar / q on sync for batch
            # 0 so the two planes relocate in parallel; gpsimd for batch 1)
            s = st8[b]
            if s.get("qkc") is None:
                s["qkc"] = qk_pool.tile([P, 2, N], BF16, tag="qkc", name="qkc")
                s["qT"] = qk_pool.tile([P, 2, N], BF16, tag="qT", name="qT")
                s["kT"] = qk_pool.tile([P, 2, N], BF16, tag="kT", name="kT")
            xT = s["xT"]
            ps = ps_small.tile([P, 512], F32, tag="smallps")
            for et in range(NT):
                nc.tensor.matmul(
                    ps, wqk_sb[:, et, t, :],
                    xT[:, et, nt * 512:(nt + 1) * 512],
                    start=(et == 0), stop=(et == NT - 1))
            nc.vector.tensor_scalar(
                out=s["qkc"][:, t, nt * 512:(nt + 1) * 512], in0=ps,
                scalar1=bqk_sb[:, t:t + 1], scalar2=None, op0=Add)
            dst = s["qT"] if t == 0 else s["kT"]
            if b == 0:
                eng = nc.scalar if t == 1 else nc.sync
            else:
                eng = nc.gpsimd
            for h in range(HEADS):
                r, c = h // 4, h % 4
                eng.dma_start(
                    out=dst[32 * c:32 * c + HD, r, nt * 512:(nt + 1) * 512],
                    in_=s["qkc"][HD * h:HD * (h + 1), t,
                                 nt * 512:(nt + 1) * 512])

        def emit_v_chunk(b, jt):
            s = st8[b]
            if s.get("v") is None:
                s["v"] = v_pool.tile([P, NT, HEADS, 32], BF16, tag="vt", name="vt")
                nc.gpsimd.memset(s["v"], 0.0)
                nc.gpsimd.memset(s["v"][:, :, :, 0:1], 1.0)
            xT = s["xT"]
            ps = ps_small.tile([P, P], F32, tag="smallps")
            for et in range(NT):
                nc.tensor.matmul(
                    ps, xT[:, et, jt * P:(jt + 1) * P], wv_sb[:, et, :],
                    start=(et == 0), stop=False)
            nc.tensor.matmul(ps, ones1_sb, bv_sb, start=False, stop=True)
            nc.vector.tensor_copy(
                out=s["v"][:, jt, :, 1:17],
                in_=ps[:].rearrange("p (h d) -> p h d", d=16))

        # ---------------- projection (single pass, both regions) --------

        def emit_proj(b, it, nt):
            s = st8[b]
            ps = ps_small.tile([P, 512], F32, tag="smallps")
            for r in range(2):
                nc.tensor.matmul(
                    ps, s["o"][r][:, it * P:(it + 1) * P],
                    wpj_sb[:, r, nt * 512:(nt + 1) * 512],
                    start=(r == 0), stop=(r == 1))
            fin = fin_pool.tile([P, 512], BF16, tag="fin")
            nc.vector.tensor_copy(out=fin, in_=ps)
            nc.sync.dma_start(
                out=out_h[b, it * P:(it + 1) * P, nt * 512:(nt + 1) * 512],
                in_=fin)

        # ---------------- attention ----------------

        def emit_normalize(b, r, ih, oT_ps):
            # oT_ps [P, 512] f32: rows 32c = softmax row sums (ones-column
            # trick).  One full-tile evacuation, reciprocal on [P,16],
            # stride-0 DRAM broadcast, then one multiply.
            s = st8[b]
            if s["o"][r] is None:
                s["o"][r] = o_pool.tile([P, N], BF16, tag="oT", name="oT")
            i0 = ih * 512
            srow = nrm_pool.tile([P, 512], F32, tag="srow")
            nc.vector.tensor_copy(out=srow, in_=oT_ps)
            scr1 = dram_pool.tile([4, 512], F32, tag="scr1")
            nc.gpsimd.dma_start(out=scr1, in_=srow[0::32, :])
            cmp = nrm_pool.tile([P, 16], F32, tag="cmp")
            flat = scr1[:].rearrange("a (pp cc) -> (a pp) cc", cc=16)
            nc.gpsimd.dma_start(out=cmp, in_=flat)
            rec = nrm_pool.tile([P, 16], F32, tag="rec")
            nc.vector.reciprocal(out=rec, in_=cmp)
            scr2 = dram_pool.tile([4, 512], F32, tag="scr2")
            nc.gpsimd.dma_start(
                out=scr2[:].rearrange("a (pp cc) -> (a pp) cc", cc=16),
                in_=rec)
            rep = nrm_pool.tile([P, 512], F32, tag="rep")
            for c in range(4):
                src = scr2[c:c + 1, :]
                bcast = bass.AP(
                    tensor=src.tensor, offset=src.offset,
                    ap=[[0, 32]] + list(src.ap[1:]))
                nc.gpsimd.dma_start(
                    out=rep[32 * c:32 * c + 32, :], in_=bcast)
            nc.vector.tensor_mul(s["o"][r][:, i0:i0 + 512], oT_ps, rep)

        def emit_attention(b, fillers):
            # group (ih, r) -- ih-major so both regions of an i-half are
            # done after two groups and the projection runs single-pass.
            # chunk (jt, cpair): 2 row-tiled K=16 scores MMs -> sc
            # [P,2,512]f32 (double-buffered) -> one N=1024 exp -> E bf16
            # -> 2 col-tiled attnv MMs accumulating into oT.
            s = st8[b]
            s["o"] = [None, None]

            def fill():
                if fillers:
                    f = fillers.pop(0)
                    if f is not None:
                        f()

            for ih in range(2):
                for r in range(2):
                    oT_ps = ps_out.tile([P, 512], F32, tag="oTps")
                    i0 = ih * 512
                    for jt in range(NT):
                        for cp in range(2):
                            c0 = 2 * cp
                            sc = ps_scores.tile([P, 2, 512], F32, tag="sc")
                            for ci in range(2):
                                c = c0 + ci
                                nc.tensor.matmul(
                                    sc[:, ci, :],
                                    s["kT"][32 * c:32 * c + HD, r,
                                            jt * P:(jt + 1) * P],
                                    s["qT"][32 * c:32 * c + HD, r,
                                            i0:i0 + 512],
                                    start=True, stop=True,
                                    tile_position=(32 * c, 0))
                            E = e_pool.tile([P, 2, 512], BF16, tag="E")
                            nc.scalar.activation(out=E, in_=sc, func=AF.Exp)
                            for ci in range(2):
                                c = c0 + ci
                                h = 4 * r + c
                                nc.tensor.matmul(
                                    oT_ps[32 * c:32 * c + 32, :],
                                    s["v"][:, jt, h, :], E[:, ci, :],
                                    start=(jt == 0), stop=(jt == NT - 1),
                                    tile_position=(0, 32 * c))
                            fill()
                    emit_normalize(b, r, ih, oT_ps)

        # ---------------- schedule ----------------
        # preload the (single) act table while the first DMAs run
        dummy = stat.tile([P, 1], F32, tag="dummy")
        nc.scalar.activation(out=dummy, in_=eps_sb, func=AF.Exp)

        # ---- batch 0 prep (the ramp): per-tile pipeline ----
        for it in range(NT):
            emit_x_load(0, it, q=it % 2)
        for it in range(NT):
            emit_ln_stats(0, it)
            emit_rstd(0, it // 4, k=it % 4)
            emit_ln_norm(0, it)
            emit_tp(0, it)
        emit_qk_chunk(0, 1, 0)   # k half 0 (scalar-queue reloc)
        emit_qk_chunk(0, 0, 0)   # q half 0 (sync-queue reloc, parallel)
        for jt in range(4):
            emit_v_chunk(0, jt)
        emit_qk_chunk(0, 1, 1)
        emit_qk_chunk(0, 0, 1)
        for jt in range(4, NT):
            emit_v_chunk(0, jt)

        # ---- fillers for attention(0): batch-1 prep + proj(0) ----
        # 64 chunks, one pop per chunk.  groups: (ih0,r0) ch 1-16,
        # (ih0,r1) 17-32, (ih1,r0) 33-48, (ih1,r1) 49-64.
        # proj(0, it0-3) valid once both ih0 normalizes land (~ch 34).
        fill_a0 = []
        for it in range(4):                                    # pops 1-4
            fill_a0.append(lambda it=it: emit_x_load(1, it, q=it % 2))
        for it in range(4):                                    # 5-8
            fill_a0.append(lambda it=it: emit_ln_stats(1, it))
        for it in range(4, NT):                                # 9-12
            fill_a0.append(lambda it=it: emit_x_load(1, it, q=it % 2))
        fill_a0.append(lambda: emit_rstd(1, 0))                # 13
        for it in range(4):                                    # 14-21
            fill_a0.append(lambda it=it: emit_ln_norm(1, it))
            fill_a0.append(lambda it=it: emit_tp(1, it))
        for it in range(4, NT):                                # 22-25
            fill_a0.append(lambda it=it: emit_ln_stats(1, it))
        fill_a0.append(lambda: emit_rstd(1, 1))                # 26
        for it in range(4, NT):                                # 27-34
            fill_a0.append(lambda it=it: emit_ln_norm(1, it))
            fill_a0.append(lambda it=it: emit_tp(1, it))
        fill_a0 += [None] * 2                                  # 35-36
        fill_a0 += [lambda: emit_qk_chunk(1, 1, 0),            # 37-40
                    lambda: emit_qk_chunk(1, 0, 0),
                    lambda: emit_qk_chunk(1, 1, 1),
                    lambda: emit_qk_chunk(1, 0, 1)]
        fill_a0 += [lambda jt=jt: emit_v_chunk(1, jt)          # 41-48
                    for jt in range(NT)]
        # proj(0, it0-3) pops 49-56 (valid from ~ch 34)
        fill_a0 += [lambda it=it, nt=nt: emit_proj(0, it, nt)
                    for it in range(4) for nt in range(2)]

        emit_attention(0, fill_a0)
        for f in fill_a0:
            if f is not None:
                f()

        # ---- attention 1 fillers ----
        fill_a1 = (
            [lambda it=it, nt=nt: emit_proj(0, it, nt)
             for it in range(4, NT) for nt in range(2)]       # pops 1-8
            + [None] * 26
            + [lambda it=it, nt=nt: emit_proj(1, it, nt)
               for it in range(4) for nt in range(2)]         # pops 35-42
        )
        emit_attention(1, fill_a1)
        for f in fill_a1:
            if f is not None:
                f()
        for it in range(4, NT):
            for nt in range(2):
                emit_proj(1, it, nt)

    nc.finalize()
    return nc


def _prep_weights(gamma, beta, w_qkv, w_proj, b_proj):
    gamma = gamma.astype(np.float64)
    beta = beta.astype(np.float64)
    w_qkv = w_qkv.astype(np.float64)
    w_proj = w_proj.astype(np.float64)
    b_proj = b_proj.astype(np.float64)

    wg = w_qkv * gamma[:, None]
    bias = beta @ w_qkv                   # [384]

    # compact q/k: tile t=0 -> q (SCALE folded), t=1 -> k
    wqk = np.zeros((EMB, 2, P), dtype=np.float64)
    wqk[:, 0, :] = wg[:, :INNER] * SCALE
    wqk[:, 1, :] = wg[:, INNER:2 * INNER]
    bqk = np.zeros((P, 2), dtype=np.float64)
    bqk[:, 0] = bias[:INNER] * SCALE
    bqk[:, 1] = bias[INNER:2 * INNER]
    wqk_t = wqk.reshape(NT, P, 2, P).transpose(1, 0, 2, 3)  # [P, NT, 2, P]

    wv = wg[:, 2 * INNER:3 * INNER].reshape(NT, P, P).transpose(1, 0, 2)
    bv = bias[2 * INNER:3 * INNER].reshape(1, P)

    # o^T row mapping: 32c = ones/rowsum row, 32c+1+d = head (4r+c) dim d
    wpj = np.zeros((P, 2, EMB), dtype=np.float64)
    for r in range(2):
        for c in range(4):
            h = 4 * r + c
            wpj[32 * c + 1:32 * c + 1 + HD, r, :] = \
                w_proj[h * HD:(h + 1) * HD, :]
    wpj[0, 0, :] = b_proj

    bf = ml_dtypes.bfloat16
    return {
        "wqk": np.ascontiguousarray(wqk_t).astype(bf),
        "bqk": np.ascontiguousarray(bqk).astype(np.float32),
        "wv": np.ascontiguousarray(wv).astype(bf),
        "bv": np.ascontiguousarray(bv).astype(bf),
        "wproj": np.ascontiguousarray(wpj).astype(bf),
    }


def kernel(x, gamma, beta, w_qkv, w_proj, b_proj):
    if "nc" not in _CACHE:
        _CACHE["nc"] = _build()
    nc = _CACHE["nc"]

    w = _prep_weights(gamma, beta, w_qkv, w_proj, b_proj)
    x = np.asarray(x, dtype=np.float32)
    in_maps = []
    for i in range(NCORES):
        m = {"xs": np.ascontiguousarray(x[i * NB:(i + 1) * NB])}
        m.update(w)
        in_maps.append(m)

    res = run_bass_kernel_spmd(nc, in_maps, core_ids=list(range(NCORES)))
    out = np.concatenate([res.results[i]["out"] for i in range(NCORES)], axis=0)
    return out.astype(np.float32)


# revision 17
# speedup vs baseline: 1.0049x; 1.0038x over previous
"""Trainium2 Bass kernel for nn_Attention_83004537963197.

LayerNorm -> QKV projection -> 8-head attention (head_dim=16) -> output
projection, x[16, 1024, 1024] f32.  Data-parallel over batch: 2 batches
per NeuronCore across 8 cores, no collectives.

v4 structure (evidence-driven, see trace analysis):
  - Software-pipelined attention chunks: scores(k+1) are EMITTED before
    attnv(k), so the in-order PE queue computes the next chunk's scores
    while exp(k) runs and the exp stream never waits on the PE
    (the naive order scores(k)->attnv(k)->scores(k+1) serializes
    attnv(k) [waits exp(k)] ahead of scores(k+1), defeating the
    double-buffered score banks).
  - One activation table set for the entire kernel: LN rstd is computed
    as exp(-0.5*ln(var+eps)) and the act-table list is filtered so Ln
    and Exp both resolve to natural_log_exp_and_others (v1 thrashed
    exp<->sqrt ACT_TABLE_LOADs mid-stream).
  - x^T via PE matmuls against identity (DMA-transpose serializes
    against all concurrent DMA traffic - measured 8us per transpose -
    so it's unusable mid-stream).
  - Attention chunk order interleaves jt-halves and regions:
    (r0,jt0-3),(r1,jt0-3),(r0,jt4-7),(r1,jt4-7) per i-half, so the
    first 16 chunks only need the nt=0 halves of q^T/k^T and attention
    starts as soon as x tiles 0-3 are normalized+transposed; the nt=1
    prep runs as fillers.
  - Both regions of an i-half finish together -> single-pass output
    projection (2 accumulating matmuls), serial tail is only the last
    i-half's 8 chunks.
  - exp N=1024 from double-buffered [P,2,512]f32 score tiles (PSUM
    banks 0-3; oT banks 4-5 (two concurrent groups); small pool 6-7),
    scores 2-way row-tiled (K=16 at 32c), attnv 2-way col-tiled,
    softmax row-sums via the ones-column trick (v_aug col 0 = 1),
    reciprocal + stride-0-DRAM-broadcast normalize with a single
    full-width PSUM evacuation.
  - Per-tile rstd during the batch-0 ramp (ScalarE idle), batched per-4
    for batch 1 (protects the exp stream).
  - Output stored bf16 (host upcasts).
"""

from contextlib import ExitStack

import numpy as np
import ml_dtypes

import concourse.bass as bass
import concourse.tile as tile
from concourse import bacc, mybir, hw_specs
from concourse.bass_utils import run_bass_kernel_spmd

# ---- single activation-table-set patch -------------------------------
_orig_get_act_tables = hw_specs.get_activation_tables


def _patched_get_act_tables(arch):
    tabs = _orig_get_act_tables(arch)
    EXP = mybir.ActivationFunctionType.Exp
    LN = mybir.ActivationFunctionType.Ln
    out = {}
    for name, funcs in tabs.items():
        if name != "natural_log_exp_and_others":
            funcs = funcs - {EXP, LN}
        out[name] = funcs
    return out


hw_specs.get_activation_tables = _patched_get_act_tables
import concourse.bacc as _bacc_mod  # noqa: E402

_bacc_mod.get_activation_tables = _patched_get_act_tables

F32 = mybir.dt.float32
BF16 = mybir.dt.bfloat16

B, N, EMB = 16, 1024, 1024
HEADS, INNER = 8, 128
HD = INNER // HEADS            # 16
SCALE = INNER ** -0.5
EPS = 1e-5
NCORES = 8
NB = B // NCORES               # batches per core
P = 128
NT = EMB // P                  # 8 tiles along emb / n

Sub = mybir.AluOpType.subtract
Mult = mybir.AluOpType.mult
Add = mybir.AluOpType.add
AF = mybir.ActivationFunctionType

_CACHE = {}


def _build():
    nc = bacc.Bacc(None, target_bir_lowering=False)

    xs_h = nc.declare_dram_parameter("xs", [NB, N, EMB], F32, isOutput=False)
    wqk_h = nc.declare_dram_parameter("wqk", [P, NT, 2, P], BF16, isOutput=False)
    bqk_h = nc.declare_dram_parameter("bqk", [P, 2], F32, isOutput=False)
    wv_h = nc.declare_dram_parameter("wv", [P, NT, P], BF16, isOutput=False)
    bv_h = nc.declare_dram_parameter("bv", [1, P], BF16, isOutput=False)
    wpj_h = nc.declare_dram_parameter("wproj", [P, 2, EMB], BF16, isOutput=False)
    id_h = nc.declare_dram_parameter("ident", [P, P], BF16, isOutput=False)
    out_h = nc.declare_dram_parameter("out", [NB, N, EMB], BF16, isOutput=True)

    with tile.TileContext(nc) as tc, ExitStack() as ctx:
        ent = ctx.enter_context
        const = ent(tc.tile_pool(name="const", bufs=1))
        xpool = ent(tc.tile_pool(name="xpool", bufs=8))
        xnpool = ent(tc.tile_pool(name="xnpool", bufs=3))
        stat = ent(tc.tile_pool(name="stat", bufs=4))
        xT_pool = ent(tc.tile_pool(name="xT", bufs=2))
        qk_pool = ent(tc.tile_pool(name="qk", bufs=2))
        v_pool = ent(tc.tile_pool(name="vp", bufs=2))
        e_pool = ent(tc.tile_pool(name="ep", bufs=4))
        o_pool = ent(tc.tile_pool(name="op", bufs=2))
        nrm_pool = ent(tc.tile_pool(name="nrm", bufs=2))
        fin_pool = ent(tc.tile_pool(name="fin", bufs=4))
        dram_pool = ent(tc.tile_pool(name="dsc", bufs=2, space="DRAM"))
        # PSUM: exactly 8 banks
        ps_scores = ent(tc.tile_pool(name="psc", bufs=2, space="PSUM"))  # 0-3
        ps_out = ent(tc.tile_pool(name="pso", bufs=2, space="PSUM"))     # 4-5
        ps_small = ent(tc.tile_pool(name="pss", bufs=2, space="PSUM"))   # 6-7

        # ---- constants (scalar queue; sync/gpsimd lead with x tiles) ----
        wqk_sb = const.tile([P, NT, 2, P], BF16)
        nc.scalar.dma_start(out=wqk_sb, in_=wqk_h[:])
        bqk_sb = const.tile([P, 2], F32)
        nc.scalar.dma_start(out=bqk_sb, in_=bqk_h[:])
        wv_sb = const.tile([P, NT, P], BF16)
        nc.scalar.dma_start(out=wv_sb, in_=wv_h[:])
        bv_sb = const.tile([1, P], BF16)
        nc.scalar.dma_start(out=bv_sb, in_=bv_h[:])
        wpj_sb = const.tile([P, 2, EMB], BF16)
        nc.scalar.dma_start(out=wpj_sb, in_=wpj_h[:])
        id_sb = const.tile([P, P], BF16)
        nc.scalar.dma_start(out=id_sb, in_=id_h[:])
        eps_sb = const.tile([P, 1], F32)
        nc.vector.memset(eps_sb, EPS)
        ones1_sb = const.tile([1, P], BF16)
        nc.vector.memset(ones1_sb, 1.0)

        st8 = {0: {}, 1: {}}   # per-batch live tiles

        # ---------------- prep: LN / transpose / qkv / v ----------------

        def _state(b):
            s = st8[b]
            if s.get("xT") is None:
                s["xT"] = xT_pool.tile([P, NT, N], BF16, tag="xTt", name="xTt")
                s["xraw"] = [None] * NT
                s["mv4"] = [None, None]
                s["rstd4"] = [None, None]
            return s

        def emit_x_load(b, it, q=0):
            s = _state(b)
            xt = xpool.tile([P, EMB], F32, tag="xt", name="xt")
            eng = nc.sync if q == 0 else nc.gpsimd
            eng.dma_start(out=xt, in_=xs_h[b, it * P:(it + 1) * P, :])
            s["xraw"][it] = xt

        def emit_ln_stats(b, it):
            s = _state(b)
            g, k = it // 4, it % 4
            if k == 0:
                s["mv4"][g] = stat.tile([P, 4, 2], F32, tag="mv4", name="mv4")
                s["rstd4"][g] = stat.tile([P, 4], F32, tag="rstd4", name="rstd4")
            xt = s["xraw"][it]
            st = stat.tile([P, 2, 6], F32, tag="st")
            nc.vector.bn_stats(out=st[:, 0, :], in_=xt[:, 0:512])
            nc.vector.bn_stats(out=st[:, 1, :], in_=xt[:, 512:1024])
            nc.vector.bn_aggr(out=s["mv4"][g][:, k, :], in_=st)

        def emit_rstd(b, g, k=None):
            # rstd = exp(-0.5*ln(var+eps)); per-tile during the batch-0
            # ramp (ScalarE idle), batched per-4 as a batch-1 filler.
            s = st8[b]
            sl = slice(0, 4) if k is None else slice(k, k + 1)
            lnv = stat.tile([P, 4], F32, tag="lnv")
            nc.scalar.activation(out=lnv[:, sl], in_=s["mv4"][g][:, sl, 1],
                                 func=AF.Ln, bias=eps_sb)
            nc.scalar.activation(out=s["rstd4"][g][:, sl], in_=lnv[:, sl],
                                 func=AF.Exp, scale=-0.5)

        def emit_ln_norm(b, it):
            s = st8[b]
            g, k = it // 4, it % 4
            xn = xnpool.tile([P, EMB], BF16, tag="xn")
            nc.vector.tensor_scalar(
                out=xn, in0=s["xraw"][it], scalar1=s["mv4"][g][:, k, 0:1],
                scalar2=s["rstd4"][g][:, k:k + 1], op0=Sub, op1=Mult)
            s["xraw"][it] = xn     # replaced by normalized bf16

        def emit_tp(b, it):
            # transpose via PE matmul against identity + DVE evacuation
            s = st8[b]
            xT = s["xT"]
            xn = s["xraw"][it]
            for eg in range(2):
                tp = ps_small.tile([P, 4, P], F32, tag="smallps")
                for kk in range(4):
                    et = 4 * eg + kk
                    nc.tensor.matmul(
                        tp[:, kk, :], xn[:, et * P:(et + 1) * P], id_sb,
                        start=True, stop=True)
                nc.vector.tensor_copy(
                    out=xT[:, 4 * eg:4 * eg + 4, it * P:(it + 1) * P],
                    in_=tp)

        def emit_qk_chunk(b, t, nt):
            # compact q^T/k^T half [128 rows = 8 heads x 16, 512 n], then
            # relocate this half's head rows into the 32-aligned region
            # layout (8 [16,512] DMAs; k on scalar / q on sync for batch 0
            # so both planes relocate in parallel; gpsimd for batch 1).
            s = st8[b]
            if s.get("qkc") is None:
                s["qkc"] = qk_pool.tile([P, 2, N], BF16, tag="qkc", name="qkc")
                s["qT"] = qk_pool.tile([P, 2, N], BF16, tag="qT", name="qT")
                s["kT"] = qk_pool.tile([P, 2, N], BF16, tag="kT", name="kT")
            xT = s["xT"]
            ps = ps_small.tile([P, 512], F32, tag="smallps")
            for et in range(NT):
                nc.tensor.matmul(
                    ps, wqk_sb[:, et, t, :],
                    xT[:, et, nt * 512:(nt + 1) * 512],
                    start=(et == 0), stop=(et == NT - 1))
            nc.vector.tensor_scalar(
                out=s["qkc"][:, t, nt * 512:(nt + 1) * 512], in0=ps,
                scalar1=bqk_sb[:, t:t + 1], scalar2=None, op0=Add)
            dst = s["qT"] if t == 0 else s["kT"]
            if b == 0:
                eng = nc.scalar if t == 1 else nc.sync
            else:
                eng = nc.gpsimd
            for h in range(HEADS):
                r, c = h // 4, h % 4
                eng.dma_start(
                    out=dst[32 * c:32 * c + HD, r, nt * 512:(nt + 1) * 512],
                    in_=s["qkc"][HD * h:HD * (h + 1), t,
                                 nt * 512:(nt + 1) * 512])

        def emit_v_chunk(b, jt):
            s = st8[b]
            if s.get("v") is None:
                s["v"] = v_pool.tile([P, NT, HEADS, 32], BF16, tag="vt", name="vt")
                nc.gpsimd.memset(s["v"], 0.0)
                nc.gpsimd.memset(s["v"][:, :, :, 0:1], 1.0)
            xT = s["xT"]
            ps = ps_small.tile([P, P], F32, tag="smallps")
            for et in range(NT):
                nc.tensor.matmul(
                    ps, xT[:, et, jt * P:(jt + 1) * P], wv_sb[:, et, :],
                    start=(et == 0), stop=False)
            nc.tensor.matmul(ps, ones1_sb, bv_sb, start=False, stop=True)
            nc.vector.tensor_copy(
                out=s["v"][:, jt, :, 1:17],
                in_=ps[:].rearrange("p (h d) -> p h d", d=16))

        # ---------------- projection (single pass, both regions) --------

        def emit_proj(b, it, nt):
            s = st8[b]
            ps = ps_small.tile([P, 512], F32, tag="smallps")
            for r in range(2):
                nc.tensor.matmul(
                    ps, s["o"][r][:, it * P:(it + 1) * P],
                    wpj_sb[:, r, nt * 512:(nt + 1) * 512],
                    start=(r == 0), stop=(r == 1))
            fin = fin_pool.tile([P, 512], BF16, tag="fin")
            nc.vector.tensor_copy(out=fin, in_=ps)
            nc.sync.dma_start(
                out=out_h[b, it * P:(it + 1) * P, nt * 512:(nt + 1) * 512],
                in_=fin)

        # ---------------- attention ----------------

        def emit_normalize(b, r, ih, oT_ps):
            # oT_ps [P, 512] f32: rows 32c = softmax row sums (ones-column
            # trick).  One full-tile evacuation, reciprocal on [P,16],
            # stride-0 DRAM broadcast, then one multiply.  Scratch hops on
            # sync (batch 0) / gpsimd (batch 1).
            s = st8[b]
            if s["o"][r] is None:
                s["o"][r] = o_pool.tile([P, N], BF16, tag="oT", name="oT")
            eng = nc.sync if b == 0 else nc.gpsimd
            i0 = ih * 512
            srow = nrm_pool.tile([P, 512], F32, tag="srow")
            nc.vector.tensor_copy(out=srow, in_=oT_ps)
            scr1 = dram_pool.tile([4, 512], F32, tag="scr1")
            eng.dma_start(out=scr1, in_=srow[0::32, :])
            cmp = nrm_pool.tile([P, 16], F32, tag="cmp")
            flat = scr1[:].rearrange("a (pp cc) -> (a pp) cc", cc=16)
            eng.dma_start(out=cmp, in_=flat)
            rec = nrm_pool.tile([P, 16], F32, tag="rec")
            nc.vector.reciprocal(out=rec, in_=cmp)
            scr2 = dram_pool.tile([4, 512], F32, tag="scr2")
            eng.dma_start(
                out=scr2[:].rearrange("a (pp cc) -> (a pp) cc", cc=16),
                in_=rec)
            rep = nrm_pool.tile([P, 512], F32, tag="rep")
            for c in range(4):
                src = scr2[c:c + 1, :]
                bcast = bass.AP(
                    tensor=src.tensor, offset=src.offset,
                    ap=[[0, 32]] + list(src.ap[1:]))
                eng.dma_start(out=rep[32 * c:32 * c + 32, :], in_=bcast)
            nc.vector.tensor_mul(s["o"][r][:, i0:i0 + 512], oT_ps, rep)

        def emit_attention(b, fillers, pops):
            # chunk list interleaves regions and jt-halves per i-half:
            #   ih: [(r0, jt0-3), (r1, jt0-3), (r0, jt4-7), (r1, jt4-7)]
            # so chunks 1-16 need only the nt=0 halves of qT/kT.
            # Scores for chunk k+1 are emitted BEFORE attnv of chunk k:
            # the PE computes them during exp(k), keeping ScalarE fed.
            s = st8[b]
            s["o"] = [None, None]
            chunks = []
            for ih in range(2):
                for half in range(2):
                    for r in range(2):
                        for jt in range(4 * half, 4 * half + 4):
                            for cp in range(2):
                                chunks.append((r, ih, jt, cp))
            nchunks = len(chunks)           # 64
            oT = {}
            sc_of = {}
            fi = [0]

            def fill(k):
                want = pops[k] if k < len(pops) else 1
                for _ in range(want):
                    if fi[0] < len(fillers):
                        f = fillers[fi[0]]
                        fi[0] += 1
                        if f is not None:
                            f()

            def emit_scores(idx):
                r, ih, jt, cp = chunks[idx]
                sc = ps_scores.tile([P, 2, 512], F32, tag="sc")
                for ci in range(2):
                    c = 2 * cp + ci
                    nc.tensor.matmul(
                        sc[:, ci, :],
                        s["kT"][32 * c:32 * c + HD, r, jt * P:(jt + 1) * P],
                        s["qT"][32 * c:32 * c + HD, r,
                                ih * 512:(ih + 1) * 512],
                        start=True, stop=True,
                        tile_position=(32 * c, 0))
                sc_of[idx] = sc

            def emit_exp_attnv(idx):
                r, ih, jt, cp = chunks[idx]
                if (r, ih) not in oT:
                    oT[(r, ih)] = ps_out.tile([P, 512], F32, tag="oTps",
                                              name="oTps")
                sc = sc_of.pop(idx)
                E = e_pool.tile([P, 2, 512], BF16, tag="E")
                nc.scalar.activation(out=E, in_=sc, func=AF.Exp)
                for ci in range(2):
                    c = 2 * cp + ci
                    h = 4 * r + c
                    nc.tensor.matmul(
                        oT[(r, ih)][32 * c:32 * c + 32, :],
                        s["v"][:, jt, h, :], E[:, ci, :],
                        start=(jt == 0), stop=(jt == NT - 1),
                        tile_position=(0, 32 * c))

            emit_scores(0)
            for k in range(nchunks):
                if k + 1 < nchunks:
                    emit_scores(k + 1)
                emit_exp_attnv(k)
                r, ih, jt, cp = chunks[k]
                if jt == NT - 1 and cp == 1:      # last chunk of (r, ih)
                    emit_normalize(b, r, ih, oT.pop((r, ih)))
                fill(k)
            while fi[0] < len(fillers):           # drain any leftovers
                f = fillers[fi[0]]
                fi[0] += 1
                if f is not None:
                    f()

        # ---------------- schedule ----------------
        # preload the (single) act table while the first DMAs run
        dummy = stat.tile([P, 1], F32, tag="dummy")
        nc.scalar.activation(out=dummy, in_=eps_sb, func=AF.Exp)

        # ---- batch 0 ramp: x loads + LN/tp for it 0-3 + nt0 qk + v0-3 --
        for it in range(NT):
            emit_x_load(0, it, q=it % 2)
        for it in range(4):
            emit_ln_stats(0, it)
            emit_rstd(0, 0, k=it)
            emit_ln_norm(0, it)
            emit_tp(0, it)
        emit_qk_chunk(0, 1, 0)   # kT half 0 (scalar-queue reloc)
        emit_qk_chunk(0, 0, 0)   # qT half 0 (sync-queue reloc, parallel)
        for jt in range(4):
            emit_v_chunk(0, jt)

        # ---- fillers for attention(0) ----
        # pops: 2/chunk for the first 16 chunks, then 1/chunk (80 total).
        pops_a0 = [2] * 16 + [1] * 48
        fill_a0 = []
        for it in range(4, NT):          # batch-0 prep tail: pops 1-20
            fill_a0.append(lambda it=it: emit_ln_stats(0, it))
            fill_a0.append(lambda it=it: emit_rstd(0, 1, k=it % 4))
            fill_a0.append(lambda it=it: emit_ln_norm(0, it))
            fill_a0.append(lambda it=it: emit_tp(0, it))
            fill_a0.append(lambda it=it: emit_v_chunk(0, it))
        fill_a0 += [lambda: emit_qk_chunk(0, 1, 1),    # 21-22
                    lambda: emit_qk_chunk(0, 0, 1)]
        for it in range(NT):                           # 23-30
            fill_a0.append(lambda it=it: emit_x_load(1, it, q=it % 2))
        for it in range(4):                            # 31-34
            fill_a0.append(lambda it=it: emit_ln_stats(1, it))
        fill_a0 += [None] * 3                          # 35-37
        fill_a0.append(lambda: emit_rstd(1, 0))        # 38
        for it in range(4):                            # 39-46
            fill_a0.append(lambda it=it: emit_ln_norm(1, it))
            fill_a0.append(lambda it=it: emit_tp(1, it))
        for it in range(4, NT):                        # 47-50
            fill_a0.append(lambda it=it: emit_ln_stats(1, it))
        fill_a0.append(lambda: emit_rstd(1, 1))        # 51
        for it in range(4, NT):                        # 52-59
            fill_a0.append(lambda it=it: emit_ln_norm(1, it))
            fill_a0.append(lambda it=it: emit_tp(1, it))
        fill_a0 += [lambda: emit_qk_chunk(1, 1, 0),    # 60-63
                    lambda: emit_qk_chunk(1, 0, 0),
                    lambda: emit_qk_chunk(1, 1, 1),
                    lambda: emit_qk_chunk(1, 0, 1)]
        fill_a0 += [lambda jt=jt: emit_v_chunk(1, jt)  # 64-71
                    for jt in range(NT)]
        # proj(0, it0-3): valid ~ch 36 (after both ih0 normalizes)
        fill_a0 += [lambda it=it, nt=nt: emit_proj(0, it, nt)   # 72-79
                    for it in range(4) for nt in range(2)]

        emit_attention(0, fill_a0, pops_a0)

        # ---- attention 1 fillers ----
        fill_a1 = (
            [None] * 8
            + [lambda it=it, nt=nt: emit_proj(0, it, nt)        # pops 9-16
               for it in range(4, NT) for nt in range(2)]
            + [None] * 32
            + [lambda it=it, nt=nt: emit_proj(1, it, nt)        # pops 49-56
               for it in range(4) for nt in range(2)]
        )
        emit_attention(1, fill_a1, [1] * 64)
        for it in range(4, NT):
            for nt in range(2):
                emit_proj(1, it, nt)

    nc.finalize()
    return nc


def _prep_weights(gamma, beta, w_qkv, w_proj, b_proj):
    gamma = gamma.astype(np.float64)
    beta = beta.astype(np.float64)
    w_qkv = w_qkv.astype(np.float64)
    w_proj = w_proj.astype(np.float64)
    b_proj = b_proj.astype(np.float64)

    wg = w_qkv * gamma[:, None]
    bias = beta @ w_qkv                   # [384]

    # compact q/k: tile t=0 -> q (SCALE folded), t=1 -> k
    wqk = np.zeros((EMB, 2, P), dtype=np.float64)
    wqk[:, 0, :] = wg[:, :INNER] * SCALE
    wqk[:, 1, :] = wg[:, INNER:2 * INNER]
    bqk = np.zeros((P, 2), dtype=np.float64)
    bqk[:, 0] = bias[:INNER] * SCALE
    bqk[:, 1] = bias[INNER:2 * INNER]
    wqk_t = wqk.reshape(NT, P, 2, P).transpose(1, 0, 2, 3)  # [P, NT, 2, P]

    wv = wg[:, 2 * INNER:3 * INNER].reshape(NT, P, P).transpose(1, 0, 2)
    bv = bias[2 * INNER:3 * INNER].reshape(1, P)

    # o^T row mapping: 32c = ones/rowsum row, 32c+1+d = head (4r+c) dim d
    wpj = np.zeros((P, 2, EMB), dtype=np.float64)
    for r in range(2):
        for c in range(4):
            h = 4 * r + c
            wpj[32 * c + 1:32 * c + 1 + HD, r, :] = \
                w_proj[h * HD:(h + 1) * HD, :]
    wpj[0, 0, :] = b_proj

    bf = ml_dtypes.bfloat16
    return {
        "wqk": np.ascontiguousarray(wqk_t).astype(bf),
        "bqk": np.ascontiguousarray(bqk).astype(np.float32),
        "wv": np.ascontiguousarray(wv).astype(bf),
        "bv": np.ascontiguousarray(bv).astype(bf),
        "wproj": np.ascontiguousarray(wpj).astype(bf),
        "ident": np.eye(P, dtype=np.float32).astype(bf),
    }


def kernel(x, gamma, beta, w_qkv, w_proj, b_proj):
    if "nc" not in _CACHE:
        _CACHE["nc"] = _build()
    nc = _CACHE["nc"]

    w = _prep_weights(gamma, beta, w_qkv, w_proj, b_proj)
    x = np.asarray(x, dtype=np.float32)
    in_maps = []
    for i in range(NCORES):
        m = {"xs": np.ascontiguousarray(x[i * NB:(i + 1) * NB])}
        m.update(w)
        in_maps.append(m)

    res = run_bass_kernel_spmd(nc, in_maps, core_ids=list(range(NCORES)))
    out = np.concatenate([res.results[i]["out"] for i in range(NCORES)], axis=0)
    return out.astype(np.float32)
